# revision 1
# baseline (speedup 1.0000x reference)
"""Trainium2 Bass kernel for nn_ClusterLoss (segment_reduce family).

Reference computes:
    loss = w0*omega_mean + (w1*omega_between + w2*omega_within) / bs
with (w0, w1, w2) = (1.0, 0.5, 0.5).

Algebra: with S_c the per-group column sums, t the total column sum vector,
B = sum_c ||S_c||^2 / max(m_c, 1):
    omega_within  = omega_mean - B
    omega_between = B - ||t||^2 / n
Since w1 == w2, B cancels exactly:
    loss = omega_mean + 0.5*(omega_mean - ||t||^2/n)/bs
so only one streaming pass over W is needed: sum of squares + row sums.
group_ids does not influence the result.

Device plan (per core, column shard [1024, 6250] f32, 8 chunks of 128 rows):
  - chunk-resident SBUF tiles [128 x 6250], filled by 1250-col DMAs
    (descending DMA widths on the last chunk so the stream tail is fine)
  - VectorE (DVE): reduce_sum along free axis -> per-partition row sums
  - ScalarE (ACT): Square activation with accum_out -> per-partition sq sums
  Engine op splits are aligned with DMA arrivals on the last chunk so the
  serial backlog after the final byte is minimal.
  - stats for chunks 0..6 are DMA'd out right behind the last w-DMAs (the
    DMA resource is exclusive, so emitting them mid-stream would displace
    the stream); only the last chunk's small stats DMA sits on the tail.
Host reduces the tiny [128, NSLOT] per-core stats in float64.
"""

import numpy as np

D = 1024
N_CLASSES = 50000
N_CORES = 8
P = 128
COLS = N_CLASSES // N_CORES      # 6250 columns per core
N_CHUNKS = D // P                # 8 partition chunks

BULK_DMA = (1250, 1250, 1250, 1250, 1250)
TAIL_DMA = (1250, 1250, 1250, 625, 625, 625, 313, 312)
BULK_ROW_OPS = (2500, 2500, 1250)
BULK_SQ_OPS = (2500, 2500, 1250)
TAIL_ROW_OPS = TAIL_DMA                       # DVE rowsums, aligned to DMAs
TAIL_SQ_ACT = (1250, 1250, 1250, 1250, 1250)  # ACT squares: fewer, coarser ops
# (ACT pays ~0.37us fixed cost per accum op, so five 1250-wide ops beat
# DMA-aligned ops in the tail; sq op k is ready once its two 625 DMAs land)
# NOTE: offloading tail squares to DVE via tensor_tensor_reduce looked good in
# the cost model but crashes real TRN2 (NRT_EXEC_UNIT_UNRECOVERABLE) in this
# toolchain even in its canonical broadcast-out form, so all squares stay on ACT.


def _widths_to_ranges(widths, off=0):
    out = []
    for w in widths:
        out.append((off, w))
        off += w
    return out


def _slots():
    """stats-column layout:
      section A (bulk, chunks 0..6): rows then sqs, BULK_OPS splits
      section B (chunk 7): rows (TAIL_ROW_OPS) | sqs (TAIL_SQ_ACT)
    Returns slots: slots[i] = (kind, chunk, off, width)."""
    slots = []
    for kind, ops in (("row", BULK_ROW_OPS), ("sq", BULK_SQ_OPS)):
        for c in range(N_CHUNKS - 1):
            for off, w in _widths_to_ranges(ops):
                slots.append((kind, c, off, w))
    bulk_len = len(slots)
    c = N_CHUNKS - 1
    for off, w in _widths_to_ranges(TAIL_ROW_OPS):
        slots.append(("row", c, off, w))
    for off, w in _widths_to_ranges(TAIL_SQ_ACT):
        slots.append(("sq", c, off, w))
    return slots, bulk_len


SLOTS, BULK_LEN = _slots()
NSLOT = len(SLOTS)
_SLOT_COL = {(k, c, off): i for i, (k, c, off, _w) in enumerate(SLOTS)}

LAST_RESULTS = None              # BassKernelResults of the most recent run
_NC_CACHE = {}


def _build_bass(bufs=3):
    import concourse.mybir as mybir
    from concourse import bacc
    from concourse.tile import TileContext

    nc = bacc.Bacc(
        "TRN2", target_bir_lowering=False, debug=False, num_devices=N_CORES
    )
    w = nc.declare_dram_parameter("w", [D, COLS], mybir.dt.float32, isOutput=False)
    out = nc.declare_dram_parameter(
        "stats", [P, NSLOT], mybir.dt.float32, isOutput=True
    )

    f32 = mybir.dt.float32
    last_c = N_CHUNKS - 1
    with TileContext(nc) as tc:
        with (
            tc.tile_pool(name="wpool", bufs=bufs) as wpool,
            tc.tile_pool(name="spool", bufs=1) as spool,
            tc.tile_pool(name="scratch", bufs=1) as scpool,
        ):
            stats = spool.tile([P, NSLOT], f32)
            scratch = scpool.tile([P, max(max(BULK_SQ_OPS), max(TAIL_SQ_ACT))], f32)
            for c in range(N_CHUNKS):
                last = c == last_c
                ctile = wpool.tile([P, COLS], f32, tag="wtile")
                for off, f in _widths_to_ranges(TAIL_DMA if last else BULK_DMA):
                    nc.sync.dma_start(
                        out=ctile[:, off:off + f],
                        in_=w[c * P:(c + 1) * P, off:off + f],
                    )
                if last:
                    # bulk stats drain in the free DMA slot behind the last
                    # w-DMAs (deps: chunks 0..6 compute, long since done).
                    # Emitting this any earlier displaces the w-stream on the
                    # exclusive DMA resource by its transfer time.
                    nc.sync.dma_start(
                        out=out[:, :BULK_LEN], in_=stats[:, :BULK_LEN]
                    )
                for off, f in _widths_to_ranges(
                    TAIL_ROW_OPS if last else BULK_ROW_OPS
                ):
                    sr = _SLOT_COL[("row", c, off)]
                    nc.vector.reduce_sum(
                        stats[:, sr:sr + 1],
                        ctile[:, off:off + f],
                        axis=mybir.AxisListType.X,
                    )
                for off, f in _widths_to_ranges(
                    TAIL_SQ_ACT if last else BULK_SQ_OPS
                ):
                    sq = _SLOT_COL[("sq", c, off)]
                    nc.scalar.activation(
                        scratch[:, :f],
                        ctile[:, off:off + f],
                        mybir.ActivationFunctionType.Square,
                        accum_out=stats[:, sq:sq + 1],
                    )
            # both engines retire their last op at ~the same time, so one
            # small SP DMA for the final-chunk stats beats per-engine DMAs
            # (each extra DMA pays its own completion latency)
            nc.sync.dma_start(
                out=out[:, BULK_LEN:], in_=stats[:, BULK_LEN:]
            )
    nc.compile()
    return nc


def kernel(softmax_weight, group_ids=None, batch_size=32, **_ignored):
    global LAST_RESULTS
    from concourse.bass_utils import run_bass_kernel_spmd

    W = np.ascontiguousarray(np.asarray(softmax_weight, dtype=np.float32))
    assert W.shape == (D, N_CLASSES), W.shape
    bs = float(np.asarray(batch_size))

    if "nc" not in _NC_CACHE:
        _NC_CACHE["nc"] = _build_bass()
    nc = _NC_CACHE["nc"]

    in_maps = [
        {"w": np.ascontiguousarray(W[:, k * COLS:(k + 1) * COLS])}
        for k in range(N_CORES)
    ]
    LAST_RESULTS = run_bass_kernel_spmd(nc, in_maps, core_ids=list(range(N_CORES)))

    om = 0.0
    t = np.zeros(D, np.float64)
    for r in LAST_RESULTS.results:
        st = r["stats"].astype(np.float64)          # [P, NSLOT]
        for i, (kind, c, _off, _w) in enumerate(SLOTS):
            if kind == "row":
                t[c * P:(c + 1) * P] += st[:, i]
            else:
                om += st[:, i].sum()

    T = (t @ t) / N_CLASSES
    loss = om + 0.5 * (om - T) / bs
    return np.asarray(loss, dtype=np.float32)



# revision 6
# speedup vs baseline: 2.9841x; 2.9841x over previous
"""Trainium2 Bass kernel for nn_ClusterLoss (segment_reduce family).

Reference computes:
    loss = w0*omega_mean + (w1*omega_between + w2*omega_within) / bs
with (w0, w1, w2) = (1.0, 0.5, 0.5).

Algebra: with S_c the per-group column sums, t the total column sum vector,
B = sum_c ||S_c||^2 / max(m_c, 1):
    omega_within  = omega_mean - B
    omega_between = B - ||t||^2 / n
Since w1 == w2, B cancels exactly:
    loss = omega_mean + 0.5*(omega_mean - ||t||^2/n)/bs
so only one streaming pass over W is needed: total sum of squares (om) and
the per-hidden-dim row sums t [1024].  group_ids does not influence the
result.

Device strategy (v2, bf16 transposed stream):
  - Host casts W to bf16 (loss tolerance is 2e-2; bf16 cast error on
    sum-of-51M-squares is ~1e-5 relative) and ships each core its column
    shard TRANSPOSED: wt [6272, 1024] bf16, rows = classes (padded with
    zeros 6250 -> 6272 = 49*128), cols = hidden dim d.  Halving the dtype
    halves DMA time; the DMA stream is the roofline.
  - SBUF tiles [128 classes, S*1024] per DMA chunk (S tiles per DMA).
  - Row sums t: PE matmul trick — lhsT = class-tile d-block [128c, 128d]
    (stationary), rhs = ones [128c, 1] (moving) -> psum[:, g] [128, 1]
    accumulated over all 49 tiles.  Moving free size is 1, so each matmul
    is ~2ns in engine time; 392 matmuls total.
  - Squares: split the free dim between ACT (Square activation with
    accum_out, one wide op per chunk) and DVE (bn_stats: 6 f32 stats per
    512-col op giving count/mean/count*var for even/odd lanes -> host
    recovers sum(x^2)).
  - Stats live in one SBUF f32 tile; bulk columns DMA out right behind the
    last w-DMA, tail columns (last chunks + psum rowsum copy) in a final
    small DMA.  Host does the tiny final reduction in float64.
"""

import numpy as np

D = 1024
N_CLASSES = 50000
N_CORES = 8
P = 128
CPC_RAW = N_CLASSES // N_CORES          # 6250 classes per core
T_TILES = (CPC_RAW + P - 1) // P        # 49 tiles of 128 classes
CPC = T_TILES * P                       # 6272 padded classes per core
G = D // P                              # 8 d-groups for rowsum matmuls

# tiles per DMA chunk; tail chunks are small so the post-stream serial
# backlog after the last byte stays minimal
CHUNKS = [4] * 11 + [2, 1, 1, 1]        # sum = 49
assert sum(CHUNKS) == T_TILES
N_TAIL_CHUNKS = 3                        # last chunks whose stats go in the
                                         # final (post-compute) stats DMA

# squares assignment per chunk: (engine, offset, width) in free-dim cols of
# the chunk's [128, S*1024] super tile.  ACT cost ~ 397 + 0.833*n (+187
# accum read); DVE bn_stats ~ 60 + 1.04*n per op (n <= 512); Pool (gpsimd)
# square ~ 95 + 1.98*n then full-reduce ~ 95 + 1.39*n.  Loads are kept at
# ~65-75% of the chunk's DMA time so the engines track the stream and the
# post-stream backlog stays small.
def _square_plan(S):
    if S == 4:
        return ([("act", 0, 1792)],
                [("dve", 1792, 512), ("dve", 2304, 512),
                 ("dve", 2816, 512), ("dve", 3328, 256)],
                [("pool", 3584, 512)])
    if S == 2:
        return ([("act", 0, 768)],
                [("dve", 768, 512), ("dve", 1280, 384)],
                [("pool", 1664, 384)])
    # single tile (tail): keep both engine chains short, no Pool (too slow
    # per element to sit on the tail latency path)
    return ([("act", 0, 384)],
            [("dve", 384, 512), ("dve", 896, 128)],
            [])


LAST_RESULTS = None              # BassKernelResults of the most recent run
_NC_CACHE = {}

# stats column bookkeeping, filled during _build_bass
_COLMAP = {"act": [], "dve": [], "pool": [], "rs": None, "bulk_len": 0,
           "nslot": 0}


def _build_bass(bufs=6):
    import concourse.mybir as mybir
    from concourse import bacc
    from concourse.tile import TileContext

    nc = bacc.Bacc(
        "TRN2", target_bir_lowering=False, debug=False, num_devices=N_CORES
    )
    f32 = mybir.dt.float32
    bf16 = mybir.dt.bfloat16

    wt = nc.declare_dram_parameter("wt", [CPC, D], bf16, isOutput=False)

    # ---- stats layout ------------------------------------------------
    # bulk section: chunks [0 .. len(CHUNKS)-1-N_TAIL_CHUNKS)
    # tail section: last N_TAIL_CHUNKS chunks' cols + 8 rowsum cols
    acts, dves, pools = [], [], []
    col = 0
    bulk_boundary_chunk = len(CHUNKS) - N_TAIL_CHUNKS
    bulk_len = None
    for ci, S in enumerate(CHUNKS):
        if ci == bulk_boundary_chunk:
            bulk_len = col
        a_ops, d_ops, p_ops = _square_plan(S)
        for _ in a_ops:
            acts.append((ci, col)); col += 1
        for _ in d_ops:
            dves.append((ci, col)); col += 6
        for _ in p_ops:
            pools.append((ci, col)); col += 1
    assert bulk_len is not None
    rs_col = col
    col += G
    nslot = col
    _COLMAP.update(act=acts, dve=dves, pool=pools, rs=rs_col,
                   bulk_len=bulk_len, nslot=nslot)

    out = nc.declare_dram_parameter("stats", [P, nslot], f32, isOutput=True)

    with TileContext(nc) as tc:
        with (
            tc.tile_pool(name="wpool", bufs=bufs) as wpool,
            tc.tile_pool(name="spool", bufs=1) as spool,
            tc.tile_pool(name="scpool", bufs=1) as scpool,
            tc.tile_pool(name="cpool", bufs=1) as cpool,
            tc.tile_pool(name="pspool", bufs=1, space="PSUM") as pspool,
        ):
            stats = spool.tile([P, nslot], f32)
            scratch = scpool.tile([P, 2048], bf16)
            pscratch = scpool.tile([P, 512], bf16)
            ones = cpool.tile([P, 1], bf16)
            nc.gpsimd.memset(ones, 1.0)
            rs_psum = pspool.tile([P, G], f32)

            a_idx = d_idx = p_idx = 0
            tile0 = 0
            for ci, S in enumerate(CHUNKS):
                ctile_full = wpool.tile([P, 4096], bf16, tag="wtile")
                ctile = ctile_full[:, :S * 1024]
                src = wt[tile0 * P:(tile0 + S) * P, :].rearrange(
                    "(s p) d -> p s d", p=P
                )
                nc.sync.dma_start(out=ctile, in_=src)
                if ci == len(CHUNKS) - 1:
                    # bulk stats drain rides right behind the last w-DMA on
                    # the (exclusive) DMA queue; its deps are long done
                    nc.sync.dma_start(
                        out=out[:, :bulk_len], in_=stats[:, :bulk_len]
                    )

                # PE rowsums: per tile s, 8 d-group matmuls, ~2ns each
                for s in range(S):
                    t_glob = tile0 + s
                    for g in range(G):
                        nc.tensor.matmul(
                            rs_psum[:, g:g + 1],
                            ctile[:, s * 1024 + g * P: s * 1024 + (g + 1) * P],
                            ones,
                            start=(t_glob == 0),
                            stop=(t_glob == T_TILES - 1),
                        )

                a_ops, d_ops, p_ops = _square_plan(S)
                for _, off, width in d_ops:
                    _, c0 = dves[d_idx]; d_idx += 1
                    nc.vector.bn_stats(stats[:, c0:c0 + 6],
                                       ctile[:, off:off + width])
                for _, off, width in p_ops:
                    _, c0 = pools[p_idx]; p_idx += 1
                    nc.gpsimd.tensor_tensor(
                        out=pscratch[:, :width],
                        in0=ctile[:, off:off + width],
                        in1=ctile[:, off:off + width],
                        op=mybir.AluOpType.mult,
                    )
                    nc.gpsimd.reduce_sum(
                        stats[0:1, c0:c0 + 1], pscratch[:, :width],
                        axis=mybir.AxisListType.XYZWC,
                    )
                for _, off, width in a_ops:
                    _, c0 = acts[a_idx]; a_idx += 1
                    nc.scalar.activation(
                        scratch[:, :width],
                        ctile[:, off:off + width],
                        mybir.ActivationFunctionType.Square,
                        accum_out=stats[:, c0:c0 + 1],
                    )
                tile0 += S

            # drain rowsum psum into the stats tile (tail), then ship the
            # tail stats section in one small DMA
            nc.scalar.copy(out=stats[:, rs_col:rs_col + G], in_=rs_psum)
            nc.sync.dma_start(out=out[:, bulk_len:], in_=stats[:, bulk_len:])
    nc.compile()
    return nc


def kernel(softmax_weight, group_ids=None, batch_size=32, **_ignored):
    global LAST_RESULTS
    import ml_dtypes
    from concourse.bass_utils import run_bass_kernel_spmd

    W = np.asarray(softmax_weight, dtype=np.float32)
    assert W.shape == (D, N_CLASSES), W.shape
    bs = float(np.asarray(batch_size))

    if "nc" not in _NC_CACHE:
        _NC_CACHE["nc"] = _build_bass()
    nc = _NC_CACHE["nc"]

    bf = ml_dtypes.bfloat16
    in_maps = []
    for k in range(N_CORES):
        shard = np.zeros((CPC, D), dtype=bf)
        shard[:CPC_RAW] = W[:, k * CPC_RAW:(k + 1) * CPC_RAW].T.astype(bf)
        in_maps.append({"wt": shard})
    LAST_RESULTS = run_bass_kernel_spmd(nc, in_maps, core_ids=list(range(N_CORES)))

    om = 0.0
    t = np.zeros(D, np.float64)
    rs_col = _COLMAP["rs"]
    for r in LAST_RESULTS.results:
        st = np.asarray(r["stats"]).astype(np.float64)   # [P, nslot]
        for _, c0 in _COLMAP["act"]:
            om += st[:, c0].sum()
        for _, c0 in _COLMAP["pool"]:
            om += st[0, c0]
        for _, c0 in _COLMAP["dve"]:
            grp = st[:, c0:c0 + 6]
            om += (grp[:, 2] + grp[:, 0] * grp[:, 1] ** 2).sum()
            om += (grp[:, 5] + grp[:, 3] * grp[:, 4] ** 2).sum()
        for g in range(G):
            t[g * P:(g + 1) * P] += st[:, rs_col + g]

    T = (t @ t) / N_CLASSES
    loss = om + 0.5 * (om - T) / bs
    return np.asarray(loss, dtype=np.float32)
